# revision 6
# baseline (speedup 1.0000x reference)
"""Trainium2 Bass kernel for the 4-layer LSTM (T=128, B=64, H=1024).

Strategy: 4-stage layer pipeline x 2-way batch data-parallel = 8 cores
(rank r: stage j = r % 4, batch half r // 4), with the per-step matmuls
in "streaming" form: the STATIONARY operand is the (tiny) transposed
activation tile xT/hT [128, 32] and the MOVING operand is the weight
matrix, 4-way column-tiled across the PE array by gate. This replaces
the baseline's 256 LDWEIGHTS-bound [128x128] weight loads per step
(~34ns each, N=32) with 64 N=512 streaming matmuls running 4-concurrent
(~3.4us/step for h@U instead of ~8.7us).

z lands in PSUM gate-major: partition 32*gate + b, free dim = 1024
h-dims. Gates run on Scalar/Vector/GpSimd with partition-shifted
reads (z_i at 0:31, z_f at 32:63, ...), cell state c lives in PSUM at
partitions 32:64 so f*c_prev reads PSUM (mixed-space operands may
differ in base partition; SBUF+SBUF pairs must match - all gate
intermediates are written to base 0). h and c are transposed back to
[128, 32] k-tile blocks via matmul-against-identity (8+8 tiny MMs/step)
to form the next step's stationary hT and the transport chunk cT.

Transport: 4-rank intra-chip AllGather ([[0-3],[4-7]]) once per 3-tick
round (6 steps) - collective-chain ops are latency-bound (~20-35us
unloaded, ~57us spacing when saturated), so one op per ~50us of compute
keeps the chain unloaded. 3-deep output rotation + 3-deep c_out
double-buffering give multi-tick safe read/write windows. Stage lag
OFF=7 ticks.

Output = cell state of layer 3 (ranks 3/7) read directly from PSUM in
[batch, hidden] layout - no host transpose.
"""

import sys

for p in ("/opt/trn_rl_repo",):
    if p not in sys.path:
        sys.path.insert(0, p)

import numpy as np
import ml_dtypes

T, B, H, L = 128, 64, 1024, 4
KT = H // 128            # 8 k-tiles over the contraction dim
B_LOC = B // 2           # batch per core
G = 2                    # steps per tick (transport granularity)
NCH = T // G             # 64 chunks per layer
R = 3                    # ticks per gather round
OFF = 7                  # tick offset between consecutive stages
C0 = 2                   # first active tick of stage 0
NTICKS = NCH + OFF * (L - 1) + C0
N_CORES = 8
CW = G * KT * B_LOC      # cols per chunk in cT layout (= 512)
SW = KT * B_LOC          # cols per step (= 256)

_CACHE = {}


def _build(nticks=NTICKS):
    import concourse.bacc as bacc
    import concourse.bass as bass
    import concourse.mybir as mybir
    import concourse.tile as tile

    bf16, f32, i32 = mybir.dt.bfloat16, mybir.dt.float32, mybir.dt.int32
    AF = mybir.ActivationFunctionType
    Alu = mybir.AluOpType

    nc = bacc.Bacc("TRN2", target_bir_lowering=False, debug=False,
                   num_devices=N_CORES)

    # weights, k-tile k at cols k*4H (moving operand layout)
    w_in = nc.dram_tensor("w_loc", [128, KT * 4 * H], bf16,
                          kind="ExternalInput")
    u_in = nc.dram_tensor("u_loc", [128, KT * 4 * H], bf16,
                          kind="ExternalInput")
    # layer-0 input, xT chunks: [128, chunk, step, k, b]
    src_static = nc.dram_tensor("src_static", [128, NCH * CW], bf16,
                                kind="ExternalInput")
    eye_in = nc.dram_tensor("eye", [32, 32], bf16, kind="ExternalInput")
    rparam = nc.dram_tensor("rparam", [1, 1], i32, kind="ExternalInput")
    out_ext = nc.dram_tensor("out", [32, H], f32, kind="ExternalOutput")

    # transport round buffers (a round = R ticks of cT chunks)
    c_out = [nc.dram_tensor(f"c_out{i}", [128, R * CW], bf16)
             for i in range(3)]
    gbufs = [nc.dram_tensor(f"gath{i}", [4, 128, R * CW], bf16)
             for i in range(3)]

    with tile.TileContext(nc) as tc:
        with (
            tc.tile_pool(name="wp", bufs=1) as wp,
            tc.tile_pool(name="sp", bufs=1) as sp,
            tc.tile_pool(name="srcp", bufs=3) as srcp,
            tc.tile_pool(name="ewp", bufs=2) as ewp,
            tc.tile_pool(name="zp", bufs=1, space="PSUM") as zp,
            tc.tile_pool(name="tp", bufs=2, space="PSUM") as tp_,
        ):
            # ---- preamble -------------------------------------------------
            w_sb = wp.tile([128, KT * 4 * H], bf16)
            u_sb = wp.tile([128, KT * 4 * H], bf16)
            nc.sync.dma_start(w_sb[:], w_in[:])
            nc.sync.dma_start(u_sb[:], u_in[:])

            eye_sb = sp.tile([32, 32], bf16)
            nc.sync.dma_start(eye_sb[:], eye_in[:])

            rp_sb = sp.tile([1, 1], i32)
            nc.sync.dma_start(rp_sb[:], rparam[:])
            rv = nc.values_load(rp_sb[:1, 0:1].to_broadcast((1, 1)))

            zsb = sp.tile([128, R * CW], bf16)
            nc.gpsimd.memset(zsb[:], 0.0)
            for cb in c_out:
                nc.sync.dma_start(cb[:, :], zsb[:])
            for gb in gbufs:
                for s in range(4):
                    nc.sync.dma_start(gb[s][:, :], zsb[:])

            # persistent state
            hT = [sp.tile([128, SW], bf16, name=f"hT{i}") for i in range(2)]
            for i in range(2):
                nc.gpsimd.memset(hT[i][:], 0.0)
            z_ps = [zp.tile([128, H], f32, name=f"z{i}") for i in range(2)]
            c_ps = zp.tile([128, H], f32, name="c_ps")
            nc.vector.memset(c_ps[32:64, :], 0.0)

            # src chunk for consuming tick tc_ (issued 2 ticks early)
            def issue_src(tc_):
                src_sb = srcp.tile([128, CW], bf16, tag="src",
                                   name=f"src_{tc_}")
                kchunk = min(max(tc_ - C0, 0), NCH - 1)
                tp = max(tc_ - OFF, 0)
                m_g = tp // R
                sub = tp % R
                sel = m_g % 3
                gt = gbufs[sel]
                with tc.If(rv == 0) as cmp:
                    nc.sync.dma_start(
                        src_sb[:],
                        src_static[:, kchunk * CW:(kchunk + 1) * CW])
                with cmp.Else():
                    for j in (1, 2, 3):
                        with tc.If(rv == j):
                            nc.sync.dma_start(
                                src_sb[:],
                                gt[j - 1][:, sub * CW:(sub + 1) * CW])
                return src_sb

            # x@W streaming matmuls for step slot s of src tile -> z buf
            def issue_xw(zt, src_t, s):
                for nh in range(2):
                    for k in range(KT):
                        for g in range(4):
                            nc.tensor.matmul(
                                zt[32 * g:32 * (g + 1),
                                   nh * 512:(nh + 1) * 512],
                                src_t[:, (s % G) * SW + 32 * k:
                                      (s % G) * SW + 32 * (k + 1)],
                                w_sb[:, k * 4 * H + g * H + nh * 512:
                                     k * 4 * H + g * H + (nh + 1) * 512],
                                start=(k == 0), stop=False,
                                skip_group_check=True,
                                tile_position=(0, 32 * g),
                            )

            def issue_hu(zt, hT_t):
                for nh in range(2):
                    for k in range(KT):
                        for g in range(4):
                            nc.tensor.matmul(
                                zt[32 * g:32 * (g + 1),
                                   nh * 512:(nh + 1) * 512],
                                hT_t[:, 32 * k:32 * (k + 1)],
                                u_sb[:, k * 4 * H + g * H + nh * 512:
                                     k * 4 * H + g * H + (nh + 1) * 512],
                                start=False,
                                stop=(k == KT - 1),
                                skip_group_check=True,
                                tile_position=(0, 32 * g),
                            )

            gstep = 0
            srcs = {0: issue_src(0), 1: issue_src(1)}
            issue_xw(z_ps[0], srcs[0], 0)

            # ---- tick loop ------------------------------------------------
            for tau in range(nticks):
                if tau % R == 0 and tau > 0:
                    m = tau // R
                    nc.gpsimd.collective_compute(
                        "AllGather", Alu.bypass,
                        replica_groups=[[0, 1, 2, 3], [4, 5, 6, 7]],
                        ins=[c_out[(m - 1) % 3].ap().opt()],
                        outs=[gbufs[(m - 1) % 3].ap().opt()],
                    )

                # state reset at each stage's first active tick
                if tau >= C0 and (tau - C0) % OFF == 0 and (tau - C0) // OFF < L:
                    j = (tau - C0) // OFF
                    with tc.If(rv == j):
                        nc.gpsimd.memset(hT[0][:], 0.0)
                        nc.gpsimd.memset(hT[1][:], 0.0)
                        nc.vector.memset(c_ps[32:64, :], 0.0)

                if tau + 2 < nticks:
                    srcs[tau + 2] = issue_src(tau + 2)
                last = tau == nticks - 1

                cbf = ewp.tile([128, CW], bf16, tag="cbf", name=f"cbf_{tau}")
                for s in range(G):
                    zt = z_ps[gstep % 2]
                    # h @ U accumulated on top of x@W prefill
                    issue_hu(zt, hT[(gstep + 1) % 2])

                    # gates (z partitions: i 0:32, f 32:64, g 64:96, o 96:128)
                    sif = ewp.tile([64, H], f32, tag="sif",
                                   name=f"sif_{tau}_{s}")
                    tg = ewp.tile([32, H], f32, tag="tg", name=f"tg_{tau}_{s}")
                    so = ewp.tile([32, H], f32, tag="so", name=f"so_{tau}_{s}")
                    nc.scalar.activation(sif[:], zt[0:64, :], AF.Sigmoid)
                    nc.scalar.activation(tg[:], zt[64:96, :], AF.Tanh)
                    nc.scalar.activation(so[:], zt[96:128, :], AF.Sigmoid)
                    fc = ewp.tile([32, H], f32, tag="fc", name=f"fc_{tau}_{s}")
                    ig = ewp.tile([32, H], f32, tag="ig", name=f"ig_{tau}_{s}")
                    nc.vector.tensor_tensor(fc[:], sif[32:64, :],
                                            c_ps[32:64, :], Alu.mult)
                    nc.gpsimd.tensor_tensor(ig[:], sif[0:32, :], tg[:],
                                            Alu.mult)
                    nc.vector.tensor_tensor(c_ps[32:64, :], fc[:], ig[:],
                                            Alu.add)
                    th = ewp.tile([32, H], f32, tag="th", name=f"th_{tau}_{s}")
                    nc.scalar.activation(th[:], c_ps[32:64, :], AF.Tanh)
                    h_bf = ewp.tile([32, H], bf16, tag="hbf",
                                    name=f"hbf_{tau}_{s}")
                    c_bf = ewp.tile([32, H], bf16, tag="cbfc",
                                    name=f"cbfc_{tau}_{s}")
                    nc.gpsimd.tensor_tensor(h_bf[:], so[:], th[:], Alu.mult)
                    nc.vector.tensor_copy(c_bf[:], c_ps[32:64, :])

                    # next step's x@W prefill streams while gates run
                    if not (last and s == G - 1):
                        ns = gstep + 1
                        nt = tau + (s + 1) // G
                        issue_xw(z_ps[ns % 2], srcs[nt], ns)

                    # transpose h and c back to [128, 32] k-tile blocks
                    ps_t = tp_.tile([128, 2 * SW], f32, tag="T",
                                    name=f"pst_{tau}_{s}")
                    for j in range(KT):
                        nc.tensor.matmul(
                            ps_t[:, 32 * j:32 * (j + 1)],
                            h_bf[:, 128 * j:128 * (j + 1)],
                            eye_sb[:],
                            start=True, stop=True, skip_group_check=True)
                    for j in range(KT):
                        nc.tensor.matmul(
                            ps_t[:, SW + 32 * j:SW + 32 * (j + 1)],
                            c_bf[:, 128 * j:128 * (j + 1)],
                            eye_sb[:],
                            start=True, stop=True, skip_group_check=True)
                    nc.vector.tensor_copy(hT[gstep % 2][:], ps_t[:, 0:SW])
                    nc.vector.tensor_copy(
                        cbf[:, s * SW:(s + 1) * SW], ps_t[:, SW:2 * SW])
                    gstep += 1

                nc.sync.dma_start(
                    c_out[(tau // R) % 3]
                    [:, (tau % R) * CW:(tau % R + 1) * CW],
                    cbf[:])
                srcs.pop(tau, None)

            # final state out
            cfin = sp.tile([32, H], f32)
            nc.vector.tensor_copy(cfin[:], c_ps[32:64, :])
            nc.sync.dma_start(out_ext[:], cfin[:])
    nc.finalize()
    return nc


def _prep_in_maps(inputs, W, U, b):
    # layer-0 xT chunks: [128, chunk, step, k, b] per batch half
    x6 = (inputs.astype(np.float32)
          .reshape(NCH, G, B, KT, 128)
          .transpose(4, 0, 1, 3, 2))       # [128, NCH, G, KT, B]
    halves = [
        np.ascontiguousarray(x6[:, :, :, :, h * B_LOC:(h + 1) * B_LOC]
                             .reshape(128, NCH * CW))
        .astype(ml_dtypes.bfloat16)
        for h in range(2)
    ]
    zeros_src = np.zeros((128, NCH * CW), dtype=ml_dtypes.bfloat16)
    # weights: k-tile k at cols k*4H (rows 128k..128k+128 of the [H,4H] mat)
    Wk = np.ascontiguousarray(
        W.reshape(L, KT, 128, 4 * H).transpose(0, 2, 1, 3)
        .reshape(L, 128, KT * 4 * H)).astype(ml_dtypes.bfloat16)
    Uk = np.ascontiguousarray(
        U.reshape(L, KT, 128, 4 * H).transpose(0, 2, 1, 3)
        .reshape(L, 128, KT * 4 * H)).astype(ml_dtypes.bfloat16)
    eye = np.eye(32, dtype=ml_dtypes.bfloat16)
    in_maps = []
    for r in range(N_CORES):
        j = r % 4
        in_maps.append({
            "w_loc": np.ascontiguousarray(Wk[j]),
            "u_loc": np.ascontiguousarray(Uk[j]),
            "src_static": halves[r // 4] if j == 0 else zeros_src,
            "eye": eye,
            "rparam": np.array([[j]], dtype=np.int32),
        })
    return in_maps


def kernel(inputs, W, U, b):
    assert not np.any(b), "nonzero bias not implemented"
    from concourse.bass_utils import run_bass_kernel_spmd

    if "nc" not in _CACHE:
        _CACHE["nc"] = _build()
    nc = _CACHE["nc"]
    in_maps = _prep_in_maps(inputs, W, U, b)
    res = run_bass_kernel_spmd(nc, in_maps, core_ids=list(range(N_CORES)))
    c = np.zeros((B, H), dtype=np.float32)
    for half, rank in ((0, 3), (1, 7)):
        c[half * B_LOC:(half + 1) * B_LOC, :] = res.results[rank]["out"]
    return c
